# revision 12
# baseline (speedup 1.0000x reference)
"""Trainium2 Bass kernel for nn_DCAM (dense transformer attention module).

Reference computation (per batch b):
  qp/kp/vp = avg_pool2d(feature_{q,k,v}, 2)            # (C=256, 64, 64)
  q = Wq @ qp, k = Wk @ kp  (M=32 channels)            # (32, N=4096)
  v = Wv @ vp                                          # (256, N)
  attn = softmax(q^T k, axis=-1)                       # (N, N)
  out[c, m] = sum_n v[c, n] attn[m, n]                 # (256, N)
  result = upsample_nearest(out, 2) + feature_v        # (256, 128, 128)

Sharding: data-parallel over batch B=8 across 8 NeuronCores (1 batch/core).

Per-core design (v2 — restructured from the hi/lo baseline):
  - All feature inputs are pre-cast to bf16 on the host; output is written
    bf16 and upcast on the host. Halves all HBM traffic.
  - q/k single bf16 (no hi/lo split): 1 S-term instead of 3. The 2e-2
    rel-err budget has ~7x slack over this.
  - The entire 2x2 sum-pooling of q/k is folded into the projection
    matmuls: 8 accumulating MMs per chunk with strided rhs APs
    (dy/dx slices of the raw 16x128 row block). No pooling DVE work at
    all on the q/k path.
  - v pooling stays a 2-step gpsimd add (from the resident fv copy);
    projection per j-block as before.
  - Phase order: fv+fk stream first (separate DMA queues) with V-pool/
    V-proj and K-proj interleaved; fq streams last and Phase B chases it
    per i-chunk, overlapping the attention with the tail of input DMA.
  - Phase B per jg: one [128,2048] S psum (4 j-blocks x 512 i), a single
    [128,2048] exp ACTIVATE (ACT does exp ONLY; all copies/evictions are
    on DVE), O-MMs per j-block/cb, and a bf16 DVE running sum for the
    softmax denominator (merged by a ones-matmul at i-chunk end).
  - softmax without max-subtraction (|s| <= ~15 fits f32/bf16 easily).
  - pooling is a 2x2 *sum*; scales fold into the exp scale (1/16) and
    into WvT (x0.25) on the host.
"""
import numpy as np
import ml_dtypes

import concourse.bass as bass
import concourse.mybir as mybir
import concourse.tile as tile
from concourse import bacc
from concourse.bass_utils import run_bass_kernel_spmd

F32 = mybir.dt.float32
BF16 = mybir.dt.bfloat16
AF = mybir.ActivationFunctionType

B = 8
C = 256
M = 32
H = W = 128
HP = WP = 64
N = HP * WP          # 4096
CB = C // 128        # 2 channel blocks
JB = N // 128        # 32 key blocks
JG = JB // 4         # 8 groups of 4 packed j-blocks
IC = N // 512        # 8 query chunks of 512


def build_module():
    nc = bacc.Bacc("TRN2", target_bir_lowering=False, debug=False)

    fq_d = nc.dram_tensor("feature_q", [C, H, W], BF16, kind="ExternalInput").ap()
    fk_d = nc.dram_tensor("feature_k", [C, H, W], BF16, kind="ExternalInput").ap()
    fv_d = nc.dram_tensor("feature_v", [C, H, W], BF16, kind="ExternalInput").ap()
    wqt_d = nc.dram_tensor("WqT", [C, M], BF16, kind="ExternalInput").ap()
    wkt_d = nc.dram_tensor("WkT", [C, M], BF16, kind="ExternalInput").ap()
    wvt_d = nc.dram_tensor("WvT", [C, C], BF16, kind="ExternalInput").ap()
    out_d = nc.dram_tensor("out", [C, H, W], BF16, kind="ExternalOutput").ap()

    with tile.TileContext(nc) as tc:
        with tc.tile_pool(name="const", bufs=1) as cpool, \
             tc.tile_pool(name="persist", bufs=1) as pp, \
             tc.tile_pool(name="ps", bufs=1, space="PSUM") as ps, \
             tc.tile_pool(name="dramb", bufs=2, space="DRAM") as dpool:
            # ---- constants ----
            wq_sb = cpool.tile([128, CB, M], BF16, name="wq")
            nc.sync.dma_start(wq_sb[:], wqt_d.rearrange("(b p) m -> p b m", p=128))
            wk_sb = cpool.tile([128, CB, M], BF16, name="wk")
            nc.sync.dma_start(wk_sb[:], wkt_d.rearrange("(b p) m -> p b m", p=128))
            wv_sb = cpool.tile([128, CB, C], BF16)
            nc.sync.dma_start(wv_sb[:], wvt_d.rearrange("(b p) c -> p b c", p=128))
            ones_b = cpool.tile([128, 1], BF16)
            nc.vector.memset(ones_b[:], 1.0)

            # ---- persistent tensors ----
            q4 = pp.tile([128, N], BF16)              # q replicated x4 groups
            k_all = pp.tile([128, JG, 128], BF16)     # [32*(jb%4)+m, jb//4, jf]
            vt_all = pp.tile([128, JB, C], BF16)      # vT[j, c] per j-block
            fv_sb = pp.tile([128, CB, H, W], BF16)    # resident residual copy

            # =========== Phase A: stream fv + fk, pool/project ===========
            # Queue assignment: fk on sync, fv on gpsimd, fq on the vector
            # queue. fq's tile rotation (bufs=3) self-throttles its stream
            # to stay just ahead of Phase B's per-i-chunk consumption, so
            # fk/fv get the HBM bandwidth first.
            def k_chunk(icn, feat, w_sb, is_q):
                xs = []
                for cb in range(CB):
                    x = pa.tile([128, 16, W], BF16, tag=f"x{'q' if is_q else 'k'}",
                                bufs=3, name="x")
                    eng = nc.gpsimd if is_q else nc.sync
                    eng.dma_start(
                        x[:], feat[cb * 128:(cb + 1) * 128,
                                   icn * 16:(icn + 1) * 16, :])
                    xs.append(x)
                pr_ps = ps.tile([128, 512], F32, tag="a", bufs=2,
                                name="pr_ps")[:M, :]
                mms = [(cb, dy, dx) for cb in range(CB)
                       for dy in range(2) for dx in range(2)]
                for mi, (cb, dy, dx) in enumerate(mms):
                    rhs = xs[cb].rearrange("c (h dy) (w dx) -> c h dy w dx",
                                           dy=2, dx=2)[:, :, dy, :, dx]
                    nc.tensor.matmul(pr_ps[:], w_sb[:, cb], rhs,
                                     start=(mi == 0), stop=(mi == len(mms) - 1),
                                     skip_group_check=True)
                cs = slice(icn * 512, (icn + 1) * 512)
                if is_q:
                    nc.vector.tensor_scalar_add(q4[0:32, cs], pr_ps[:], 0.0)
                    for g in range(1, 4):
                        nc.sync.dma_start(q4[g * 32:(g + 1) * 32, cs],
                                          q4[0:32, cs])
                else:
                    for t in range(4):
                        nc.vector.tensor_scalar_add(
                            k_all[t * 32:(t + 1) * 32, icn, :],
                            pr_ps[:, t * 128:(t + 1) * 128], 0.0)

            with tc.tile_pool(name="poolA", bufs=1) as pa:
                def v_pool(slab):
                    r0 = slab * 32
                    vph = pa.tile([128, CB, 16, WP], BF16, tag="vph", bufs=2,
                                  name="vph")
                    for cb in range(CB):
                        src = fv_sb[:, cb, r0:r0 + 32, :].rearrange(
                            "c (h dy) (w dx) -> c h dy w dx", dy=2, dx=2)
                        rfv = pa.tile([128, 16, WP, 2], BF16, tag="rfv",
                                      bufs=2, name="rfv")
                        nc.vector.tensor_add(rfv[:], src[:, :, 0], src[:, :, 1])
                        nc.gpsimd.tensor_add(vph[:, cb], rfv[:, :, :, 0],
                                             rfv[:, :, :, 1])
                    return vph

                def v_proj(slab, vph):
                    for r2 in range(8):   # j-blocks in this slab
                        jb = slab * 8 + r2
                        vt_ps = ps.tile([128, 512], F32, tag="a",
                                        bufs=2, name="vt_ps")[:, :C]
                        nc.tensor.matmul(vt_ps[:],
                                         vph[:, 0, r2 * 2:r2 * 2 + 2, :],
                                         wv_sb[:, 0], start=True, stop=False)
                        nc.tensor.matmul(vt_ps[:],
                                         vph[:, 1, r2 * 2:r2 * 2 + 2, :],
                                         wv_sb[:, 1], start=False, stop=True)
                        nc.vector.tensor_scalar_add(vt_all[:, jb, :],
                                                    vt_ps[:], 0.0)

                # fv streams on the (otherwise idle until B) scalar queue
                for slab in range(4):
                    r0 = slab * 32
                    for cb in range(CB):
                        nc.scalar.dma_start(
                            fv_sb[:, cb, r0:r0 + 32, :],
                            fv_d[cb * 128:(cb + 1) * 128, r0:r0 + 32, :])
                # slabs 0/1 fully in phase A; slabs 2/3 pool here but their
                # projections are deferred into B(ic0)'s jg loop, so the PE
                # FIFO reaches the first attention matmuls early.
                deferred = {}
                for slab in range(4):
                    vph = v_pool(slab)
                    if slab < 2:
                        v_proj(slab, vph)
                    else:
                        deferred[slab] = vph
                    for icn in (slab * 2, slab * 2 + 1):
                        k_chunk(icn, fk_d, wk_sb, is_q=False)

                # ===== Phase B: attention, emitted inline per i-chunk =====
                # The PE queue is strict FIFO, so B(ic) is emitted right
                # after q-proj(ic): attention for chunk 0 starts as soon as
                # fk/fv + the first fq chunk have landed, and the tail of
                # the fq stream overlaps the attention steady state.
                # Software-pipelined by one jg: the O matmuls for jg-1 are
                # issued after exp(jg), so the PE streams O(jg-1) while ACT
                # computes exp(jg) - neither engine waits on the other, and
                # the PE stays HAM-warm.
                pb = pa

                def o_mms(ic, jg, p):
                    for t in range(4):
                        j = jg * 4 + t
                        pr = p[:, t * 512:(t + 1) * 512]
                        for cb in range(CB):
                            nc.tensor.matmul(
                                o_ps[cb][:],
                                vt_all[:, j, cb * 128:(cb + 1) * 128],
                                pr,
                                start=(j == 0), stop=(j == JB - 1),
                                skip_group_check=True)

                for ic in range(IC):
                    k_chunk(ic, fq_d, wq_sb, is_q=True)
                    i0 = ic * 512
                    lacc = pb.tile([128, 2048], BF16, tag="lacc", bufs=2,
                                   name="lacc")
                    o_ps = [ps.tile([128, 512], F32, tag=f"o{cb}", bufs=1,
                                    name=f"o{cb}_ps")
                            for cb in range(CB)]
                    p_prev = None
                    for jg in range(JG):
                        # late v-projections for the deferred slabs, placed
                        # just ahead of the o_mms that first need them
                        if ic == 0 and jg in (4, 6):
                            v_proj(jg // 2, deferred[jg // 2])
                        s_ps = ps.tile([128, 2048], F32, tag="s", bufs=1,
                                       name="s_ps")
                        # 16-way (32x32) row+col tiling: all 16 MMs run
                        # concurrently in distinct PE sub-arrays.
                        for t in range(4):
                            gs = slice(t * 32, (t + 1) * 32)
                            for u in range(4):
                                nc.tensor.matmul(
                                    s_ps[u * 32:(u + 1) * 32,
                                         t * 512:(t + 1) * 512],
                                    k_all[gs, jg, u * 32:(u + 1) * 32],
                                    q4[gs, i0:i0 + 512],
                                    start=True, stop=True,
                                    tile_position=(t * 32, u * 32),
                                    skip_group_check=True)
                        p = pb.tile([128, 2048], BF16, tag="p", bufs=4,
                                    name="p")
                        nc.scalar.activation(p[:], s_ps[:], AF.Exp,
                                             scale=0.0625)
                        if p_prev is not None:
                            o_mms(ic, jg - 1, p_prev)
                        if jg == 0:
                            nc.vector.tensor_scalar_add(lacc[:], p[:], 0.0)
                        else:
                            nc.vector.tensor_add(lacc[:], lacc[:], p[:])
                        p_prev = p
                    o_mms(ic, JG - 1, p_prev)
                    # ---- epilogue: fully async off the jg pipeline ----
                    # l first (its copy releases the s psum tag for the next
                    # i-chunk), then the o evictions release the o banks;
                    # 1/l + upsample+residual run from SBUF behind the scenes.
                    l_ps = ps.tile([128, 2048], F32, tag="s", bufs=1,
                                   name="l_ps")
                    for t in range(4):
                        nc.tensor.matmul(l_ps[:1, :512], ones_b[:],
                                         lacc[:, t * 512:(t + 1) * 512],
                                         start=(t == 0), stop=(t == 3),
                                         skip_group_check=True)
                    l_sb = pb.tile([1, 512], F32, tag="l_sb", bufs=2,
                                   name="l_sb")
                    nc.vector.tensor_scalar_add(l_sb[:], l_ps[:1, :512], 0.0)
                    o_sb = []
                    for cb in range(CB):
                        t_ = pb.tile([128, 512], F32, tag=f"osb{cb}", bufs=2,
                                     name=f"osb{cb}")
                        nc.vector.tensor_scalar_add(t_[:], o_ps[cb][:], 0.0)
                        o_sb.append(t_)
                    # 1/l on a (128, 4) transposed view (DRAM bounce), then
                    # broadcast back across partitions.
                    l_dr = dpool.tile([512], F32, tag="l_dr", bufs=2,
                                      name="l_dr")
                    nc.sync.dma_start(l_dr[:], l_sb[:])
                    lT = pb.tile([128, 4], F32, tag="lT", bufs=2, name="lT")
                    nc.sync.dma_start(lT[:], l_dr.rearrange("(p b) -> p b",
                                                            b=4))
                    rT = pb.tile([128, 4], F32, tag="rT", bufs=2, name="rT")
                    nc.vector.reciprocal(rT[:], lT[:])
                    r_dr = dpool.tile([512], F32, tag="r_dr", bufs=2,
                                      name="r_dr")
                    nc.sync.dma_start(r_dr.rearrange("(p b) -> p b", b=4),
                                      rT[:])
                    rb = pb.tile([128, 512], F32, tag="rb", bufs=2,
                                 name="rb")
                    nc.sync.dma_start(
                        rb[:],
                        r_dr.rearrange("(o x) -> o x", o=1).to_broadcast(
                            (128, 512)))
                    for cb in range(CB):
                        oc = pb.tile([128, 512], BF16, tag="oc", bufs=2,
                                     name="oc")
                        nc.vector.tensor_mul(oc[:], o_sb[cb][:], rb[:])
                        final = pb.tile([128, 8, 2, WP, 2], BF16, tag="final",
                                        bufs=3, name="final")
                        up = oc.rearrange("c (h w) -> c h w", w=WP)[
                            :, :, :, None].to_broadcast((128, 8, WP, 2))
                        fvv = fv_sb[:, cb, ic * 16:(ic + 1) * 16, :].rearrange(
                            "c (h dy) (w dx) -> c h dy w dx", dy=2, dx=2)
                        nc.vector.tensor_add(final[:, :, 0], up, fvv[:, :, 0])
                        nc.vector.tensor_add(final[:, :, 1], up, fvv[:, :, 1])
                        nc.sync.dma_start(
                            out_d[cb * 128:(cb + 1) * 128,
                                  ic * 16:(ic + 1) * 16, :],
                            final.rearrange("c h dy w dx -> c (h dy) (w dx)"))

    nc.compile()
    return nc


_NC_CACHE = []
LAST_RESULT = []  # last BassKernelResults, for perf inspection by test.py


def kernel(**inputs) -> np.ndarray:
    fq = np.ascontiguousarray(
        np.asarray(inputs["feature_q"], dtype=np.float32).astype(
            ml_dtypes.bfloat16))
    fk = np.ascontiguousarray(
        np.asarray(inputs["feature_k"], dtype=np.float32).astype(
            ml_dtypes.bfloat16))
    fv = np.ascontiguousarray(
        np.asarray(inputs["feature_v"], dtype=np.float32).astype(
            ml_dtypes.bfloat16))
    wq = np.asarray(inputs["Wq"], dtype=np.float32)
    wk = np.asarray(inputs["Wk"], dtype=np.float32)
    wv = np.asarray(inputs["Wv"], dtype=np.float32)

    # weight layout prep (pure layout/scale folding, no heavy compute):
    # on-device pooling is a 2x2 *sum*; q,k each pick up 4x -> s is 16x,
    # folded into the on-device exp scale; v's 4x is folded into WvT here.
    wqt = np.ascontiguousarray(wq.T.astype(ml_dtypes.bfloat16))
    wkt = np.ascontiguousarray(wk.T.astype(ml_dtypes.bfloat16))
    wvt = np.ascontiguousarray(
        (wv.T * 0.25).astype(ml_dtypes.bfloat16))     # (C, C) [c_in, c_out]

    if not _NC_CACHE:
        _NC_CACHE.append(build_module())
    nc = _NC_CACHE[0]

    in_maps = [
        {
            "feature_q": fq[b],
            "feature_k": fk[b],
            "feature_v": fv[b],
            "WqT": wqt,
            "WkT": wkt,
            "WvT": wvt,
        }
        for b in range(B)
    ]
    res = run_bass_kernel_spmd(nc, in_maps, core_ids=list(range(B)))
    LAST_RESULT.clear()
    LAST_RESULT.append(res)
    out = np.stack([np.asarray(res.results[b]["out"]) for b in range(B)],
                   axis=0)
    return out.astype(np.float32)


if __name__ == "__main__":
    nc = build_module()
    print("module built + compiled OK")


# revision 18
# speedup vs baseline: 1.2476x; 1.2476x over previous
"""Trainium2 Bass kernel for nn_DCAM (dense transformer attention module).

Reference computation (per batch b):
  qp/kp/vp = avg_pool2d(feature_{q,k,v}, 2)            # (C=256, 64, 64)
  q = Wq @ qp, k = Wk @ kp  (M=32 channels)            # (32, N=4096)
  v = Wv @ vp                                          # (256, N)
  attn = softmax(q^T k, axis=-1)                       # (N, N)
  out[c, m] = sum_n v[c, n] attn[m, n]                 # (256, N)
  result = upsample_nearest(out, 2) + feature_v        # (256, 128, 128)

Sharding: data-parallel over batch B=8 across 8 NeuronCores (1 batch/core).

Per-core design (v2 — restructured from the hi/lo baseline):
  - All feature inputs are pre-cast to bf16 on the host; output is written
    bf16 and upcast on the host. Halves all HBM traffic.
  - q/k single bf16 (no hi/lo split): 1 S-term instead of 3. The 2e-2
    rel-err budget has ~7x slack over this.
  - The entire 2x2 sum-pooling of q/k is folded into the projection
    matmuls: 8 accumulating MMs per chunk with strided rhs APs
    (dy/dx slices of the raw 16x128 row block). No pooling DVE work at
    all on the q/k path.
  - v pooling stays a 2-step gpsimd add (from the resident fv copy);
    projection per j-block as before.
  - Phase order: fv+fk stream first (separate DMA queues) with V-pool/
    V-proj and K-proj interleaved; fq streams last and Phase B chases it
    per i-chunk, overlapping the attention with the tail of input DMA.
  - Phase B per jg: one [128,2048] S psum (4 j-blocks x 512 i), a single
    [128,2048] exp ACTIVATE (ACT does exp ONLY; all copies/evictions are
    on DVE), O-MMs per j-block/cb, and a bf16 DVE running sum for the
    softmax denominator (merged by a ones-matmul at i-chunk end).
  - softmax without max-subtraction (|s| <= ~15 fits f32/bf16 easily).
  - pooling is a 2x2 *sum*; scales fold into the exp scale (1/16) and
    into WvT (x0.25) on the host.
"""
import numpy as np
import ml_dtypes

import concourse.bass as bass
import concourse.mybir as mybir
import concourse.tile as tile
from concourse import bacc
from concourse.bass_utils import run_bass_kernel_spmd

F32 = mybir.dt.float32
BF16 = mybir.dt.bfloat16
AF = mybir.ActivationFunctionType

B = 8
C = 256
M = 32
H = W = 128
HP = WP = 64
N = HP * WP          # 4096
CB = C // 128        # 2 channel blocks
JB = N // 128        # 32 key blocks
JG = JB // 4         # 8 groups of 4 packed j-blocks
IC = N // 512        # 8 query chunks of 512


def build_module():
    nc = bacc.Bacc("TRN2", target_bir_lowering=False, debug=False)

    fq_d = nc.dram_tensor("feature_q", [C, H, W], BF16, kind="ExternalInput").ap()
    fk_d = nc.dram_tensor("feature_k", [C, H, W], BF16, kind="ExternalInput").ap()
    fv_d = nc.dram_tensor("feature_v", [C, H, W], BF16, kind="ExternalInput").ap()
    wqt_d = nc.dram_tensor("WqT", [C, M], BF16, kind="ExternalInput").ap()
    wkt_d = nc.dram_tensor("WkT", [C, M], BF16, kind="ExternalInput").ap()
    wvt_d = nc.dram_tensor("WvT", [C, C], BF16, kind="ExternalInput").ap()
    out_d = nc.dram_tensor("out", [C, H, W], BF16, kind="ExternalOutput").ap()

    with tile.TileContext(nc) as tc:
        with tc.tile_pool(name="const", bufs=1) as cpool, \
             tc.tile_pool(name="persist", bufs=1) as pp, \
             tc.tile_pool(name="ps", bufs=1, space="PSUM") as ps, \
             tc.tile_pool(name="dramb", bufs=2, space="DRAM") as dpool:
            # ---- constants ----
            wq_sb = cpool.tile([128, CB, M], BF16, name="wq")
            nc.sync.dma_start(wq_sb[:], wqt_d.rearrange("(b p) m -> p b m", p=128))
            wk_sb = cpool.tile([128, CB, M], BF16, name="wk")
            nc.sync.dma_start(wk_sb[:], wkt_d.rearrange("(b p) m -> p b m", p=128))
            wv_sb = cpool.tile([128, CB, C], BF16)
            nc.sync.dma_start(wv_sb[:], wvt_d.rearrange("(b p) c -> p b c", p=128))
            ones_b = cpool.tile([128, 1], BF16)
            nc.vector.memset(ones_b[:], 1.0)

            # ---- persistent tensors ----
            q4 = pp.tile([128, N], BF16)              # q replicated x4 groups
            k_all = pp.tile([128, JG, 128], BF16)     # [32*(jb%4)+m, jb//4, jf]
            vt_all = pp.tile([128, JB, C], BF16)      # vT[j, c] per j-block
            fv_sb = pp.tile([128, CB, H, W], BF16)    # resident residual copy

            # =========== Phase A: stream fv + fk, pool/project ===========
            # Queue assignment: fk on sync, fv on gpsimd, fq on the vector
            # queue. fq's tile rotation (bufs=3) self-throttles its stream
            # to stay just ahead of Phase B's per-i-chunk consumption, so
            # fk/fv get the HBM bandwidth first.
            def k_chunk(icn, feat, w_sb, is_q):
                xs = []
                for cb in range(CB):
                    x = pa.tile([128, 16, W], BF16, tag=f"x{'q' if is_q else 'k'}",
                                bufs=6 if not is_q else 3, name="x")
                    eng = nc.gpsimd if is_q else nc.sync
                    eng.dma_start(
                        x[:], feat[cb * 128:(cb + 1) * 128,
                                   icn * 16:(icn + 1) * 16, :])
                    xs.append(x)
                pr_ps = ps.tile([128, 512], F32, tag="a", bufs=2,
                                name="pr_ps")[:M, :]
                mms = [(cb, dy, dx) for cb in range(CB)
                       for dy in range(2) for dx in range(2)]
                for mi, (cb, dy, dx) in enumerate(mms):
                    rhs = xs[cb].rearrange("c (h dy) (w dx) -> c h dy w dx",
                                           dy=2, dx=2)[:, :, dy, :, dx]
                    nc.tensor.matmul(pr_ps[:], w_sb[:, cb], rhs,
                                     start=(mi == 0), stop=(mi == len(mms) - 1),
                                     skip_group_check=True)
                cs = slice(icn * 512, (icn + 1) * 512)
                if is_q:
                    nc.vector.tensor_scalar_add(q4[0:32, cs], pr_ps[:], 0.0)
                    for g in range(1, 4):
                        nc.gpsimd.dma_start(q4[g * 32:(g + 1) * 32, cs],
                                            q4[0:32, cs])
                else:
                    for t in range(4):
                        nc.vector.tensor_scalar_add(
                            k_all[t * 32:(t + 1) * 32, icn, :],
                            pr_ps[:, t * 128:(t + 1) * 128], 0.0)

            with tc.tile_pool(name="poolA", bufs=1) as pa:
                def v_pool(slab):
                    r0 = slab * 32
                    vph = pa.tile([128, CB, 16, WP], BF16, tag="vph", bufs=2,
                                  name="vph")
                    for cb in range(CB):
                        src = fv_sb[:, cb, r0:r0 + 32, :].rearrange(
                            "c (h dy) (w dx) -> c h dy w dx", dy=2, dx=2)
                        rfv = pa.tile([128, 16, WP, 2], BF16, tag="rfv",
                                      bufs=2, name="rfv")
                        nc.vector.tensor_add(rfv[:], src[:, :, 0], src[:, :, 1])
                        nc.gpsimd.tensor_add(vph[:, cb], rfv[:, :, :, 0],
                                             rfv[:, :, :, 1])
                    return vph

                def v_proj(slab, vph):
                    for r2 in range(8):   # j-blocks in this slab
                        jb = slab * 8 + r2
                        vt_ps = ps.tile([128, 512], F32, tag="a",
                                        bufs=2, name="vt_ps")[:, :C]
                        nc.tensor.matmul(vt_ps[:],
                                         vph[:, 0, r2 * 2:r2 * 2 + 2, :],
                                         wv_sb[:, 0], start=True, stop=False)
                        nc.tensor.matmul(vt_ps[:],
                                         vph[:, 1, r2 * 2:r2 * 2 + 2, :],
                                         wv_sb[:, 1], start=False, stop=True)
                        nc.vector.tensor_scalar_add(vt_all[:, jb, :],
                                                    vt_ps[:], 0.0)

                # fv loads lead the sync queue (ahead of all fk chunks), so
                # vt_all is complete before the attention needs it; the k
                # projections are deferred into B(ic0)'s jg loop and chase
                # the fk stream chunk by chunk.
                for slab in range(4):
                    r0 = slab * 32
                    for cb in range(CB):
                        nc.sync.dma_start(
                            fv_sb[:, cb, r0:r0 + 32, :],
                            fv_d[cb * 128:(cb + 1) * 128, r0:r0 + 32, :])
                for slab in range(4):
                    v_proj(slab, v_pool(slab))

                # ===== Phase B: attention, emitted inline per i-chunk =====
                # The PE queue is strict FIFO, so B(ic) is emitted right
                # after q-proj(ic): attention for chunk 0 starts as soon as
                # fk/fv + the first fq chunk have landed, and the tail of
                # the fq stream overlaps the attention steady state.
                # Software-pipelined by one jg: the O matmuls for jg-1 are
                # issued after exp(jg), so the PE streams O(jg-1) while ACT
                # computes exp(jg) - neither engine waits on the other, and
                # the PE stays HAM-warm.
                pb = pa

                def o_mms(ic, jg, p):
                    for t in range(4):
                        j = jg * 4 + t
                        pr = p[:, t * 512:(t + 1) * 512]
                        for cb in range(CB):
                            nc.tensor.matmul(
                                o_ps[cb][:],
                                vt_all[:, j, cb * 128:(cb + 1) * 128],
                                pr,
                                start=(j == 0), stop=(j == JB - 1),
                                skip_group_check=True)

                for ic in range(IC):
                    k_chunk(ic, fq_d, wq_sb, is_q=True)
                    i0 = ic * 512
                    lacc = pb.tile([128, 2048], BF16, tag="lacc", bufs=2,
                                   name="lacc")
                    o_ps = [ps.tile([128, 512], F32, tag=f"o{cb}", bufs=1,
                                    name=f"o{cb}_ps")
                            for cb in range(CB)]
                    p_prev = None
                    for jg in range(JG):
                        if ic == 0:
                            # deferred k projection: k(jg) just before the
                            # S matmuls that consume it
                            k_chunk(jg, fk_d, wk_sb, is_q=False)
                        s_ps = ps.tile([128, 2048], F32, tag="s", bufs=1,
                                       name="s_ps")
                        for t in range(4):
                            gs = slice(t * 32, (t + 1) * 32)
                            nc.tensor.matmul(
                                s_ps[:, t * 512:(t + 1) * 512],
                                k_all[gs, jg, :], q4[gs, i0:i0 + 512],
                                start=True, stop=True,
                                tile_position=(t * 32, 0),
                                skip_group_check=True)
                        p = pb.tile([128, 2048], BF16, tag="p", bufs=4,
                                    name="p")
                        nc.scalar.activation(p[:], s_ps[:], AF.Exp,
                                             scale=0.0625)
                        if p_prev is not None:
                            o_mms(ic, jg - 1, p_prev)
                        if jg == 0:
                            nc.vector.tensor_scalar_add(lacc[:], p[:], 0.0)
                        else:
                            nc.vector.tensor_add(lacc[:], lacc[:], p[:])
                        p_prev = p
                    o_mms(ic, JG - 1, p_prev)
                    # ---- epilogue: fully async off the jg pipeline ----
                    # l first (its copy releases the s psum tag for the next
                    # i-chunk), then the o evictions release the o banks;
                    # 1/l + upsample+residual run from SBUF behind the scenes.
                    l_ps = ps.tile([128, 2048], F32, tag="s", bufs=1,
                                   name="l_ps")
                    for t in range(4):
                        nc.tensor.matmul(l_ps[:1, :512], ones_b[:],
                                         lacc[:, t * 512:(t + 1) * 512],
                                         start=(t == 0), stop=(t == 3),
                                         skip_group_check=True)
                    l_sb = pb.tile([1, 512], F32, tag="l_sb", bufs=2,
                                   name="l_sb")
                    nc.vector.tensor_scalar_add(l_sb[:], l_ps[:1, :512], 0.0)
                    o_sb = []
                    for cb in range(CB):
                        t_ = pb.tile([128, 512], F32, tag=f"osb{cb}", bufs=2,
                                     name=f"osb{cb}")
                        nc.vector.tensor_scalar_add(t_[:], o_ps[cb][:], 0.0)
                        o_sb.append(t_)
                    # 1/l on a (128, 4) transposed view (DRAM bounce), then
                    # broadcast back across partitions.
                    l_dr = dpool.tile([512], F32, tag="l_dr", bufs=2,
                                      name="l_dr")
                    nc.sync.dma_start(l_dr[:], l_sb[:])
                    lT = pb.tile([128, 4], F32, tag="lT", bufs=2, name="lT")
                    nc.sync.dma_start(lT[:], l_dr.rearrange("(p b) -> p b",
                                                            b=4))
                    rT = pb.tile([128, 4], F32, tag="rT", bufs=2, name="rT")
                    nc.vector.reciprocal(rT[:], lT[:])
                    r_dr = dpool.tile([512], F32, tag="r_dr", bufs=2,
                                      name="r_dr")
                    nc.sync.dma_start(r_dr.rearrange("(p b) -> p b", b=4),
                                      rT[:])
                    rb = pb.tile([128, 512], F32, tag="rb", bufs=2,
                                 name="rb")
                    nc.sync.dma_start(
                        rb[:],
                        r_dr.rearrange("(o x) -> o x", o=1).to_broadcast(
                            (128, 512)))
                    for cb in range(CB):
                        oc = pb.tile([128, 512], BF16, tag="oc", bufs=2,
                                     name="oc")
                        nc.vector.tensor_mul(oc[:], o_sb[cb][:], rb[:])
                        final = pb.tile([128, 8, 2, WP, 2], BF16, tag="final",
                                        bufs=3, name="final")
                        up = oc.rearrange("c (h w) -> c h w", w=WP)[
                            :, :, :, None].to_broadcast((128, 8, WP, 2))
                        fvv = fv_sb[:, cb, ic * 16:(ic + 1) * 16, :].rearrange(
                            "c (h dy) (w dx) -> c h dy w dx", dy=2, dx=2)
                        nc.vector.tensor_add(final[:, :, 0], up, fvv[:, :, 0])
                        nc.vector.tensor_add(final[:, :, 1], up, fvv[:, :, 1])
                        nc.sync.dma_start(
                            out_d[cb * 128:(cb + 1) * 128,
                                  ic * 16:(ic + 1) * 16, :],
                            final.rearrange("c h dy w dx -> c (h dy) (w dx)"))

    nc.compile()
    return nc


_NC_CACHE = []
LAST_RESULT = []  # last BassKernelResults, for perf inspection by test.py


def kernel(**inputs) -> np.ndarray:
    fq = np.ascontiguousarray(
        np.asarray(inputs["feature_q"], dtype=np.float32).astype(
            ml_dtypes.bfloat16))
    fk = np.ascontiguousarray(
        np.asarray(inputs["feature_k"], dtype=np.float32).astype(
            ml_dtypes.bfloat16))
    fv = np.ascontiguousarray(
        np.asarray(inputs["feature_v"], dtype=np.float32).astype(
            ml_dtypes.bfloat16))
    wq = np.asarray(inputs["Wq"], dtype=np.float32)
    wk = np.asarray(inputs["Wk"], dtype=np.float32)
    wv = np.asarray(inputs["Wv"], dtype=np.float32)

    # weight layout prep (pure layout/scale folding, no heavy compute):
    # on-device pooling is a 2x2 *sum*; q,k each pick up 4x -> s is 16x,
    # folded into the on-device exp scale; v's 4x is folded into WvT here.
    wqt = np.ascontiguousarray(wq.T.astype(ml_dtypes.bfloat16))
    wkt = np.ascontiguousarray(wk.T.astype(ml_dtypes.bfloat16))
    wvt = np.ascontiguousarray(
        (wv.T * 0.25).astype(ml_dtypes.bfloat16))     # (C, C) [c_in, c_out]

    if not _NC_CACHE:
        _NC_CACHE.append(build_module())
    nc = _NC_CACHE[0]

    in_maps = [
        {
            "feature_q": fq[b],
            "feature_k": fk[b],
            "feature_v": fv[b],
            "WqT": wqt,
            "WkT": wkt,
            "WvT": wvt,
        }
        for b in range(B)
    ]
    res = run_bass_kernel_spmd(nc, in_maps, core_ids=list(range(B)))
    LAST_RESULT.clear()
    LAST_RESULT.append(res)
    out = np.stack([np.asarray(res.results[b]["out"]) for b in range(B)],
                   axis=0)
    return out.astype(np.float32)


if __name__ == "__main__":
    nc = build_module()
    print("module built + compiled OK")
